# revision 1
# baseline (speedup 1.0000x reference)
"""Trainium2 Bass device kernel for nn_KATRec (GCN + transformer + logits).

8-core SPMD plan (full I/O):
  L1 GCN: row-sharded (15104 slots/core, degree-sorted chunks); indirect-DMA
    gathers of fp16 x0 rows; DVE broadcast-mul + strided reduce; x1 shard
    resident in SBUF f32, cast to fp16 on DMA-out; AllGather -> X1TAB.
  L2 GCN: per-core "needed entities" only (token entities of its batch shard
    + item entities of its vocab shard), degree-sorted chunks gathering from
    X1TAB; x0/x1 self rows appended as extra edges with val 1/3 so the chunk
    reduce directly yields s = (x0+x1+x2)/3; s table -> local DRAM fp16.
  Transformer: batch-sharded 64 seqs/core, L padded to 64 (2 seqs per
    128-partition tile, 32 tiles); KG attention bias built on device from
    s-gathers; causal/pad masks precomputed host-side as fp16 mul/add tiles.
  Logits: vocab-sharded 6272-padded items/core; item vectors built
    transposed [64, 6272]; user_vec AllGather; out block [512, 6250]/core.
"""

import contextlib

import numpy as np

import concourse.bass as bass
import concourse.mybir as mybir
import concourse.tile as tile
from concourse import library_config
from concourse.masks import make_identity

# ---- problem constants ----
NUM_ITEMS = 50000
NUM_USERS = 20000
NUM_ENTITIES = 100000
N_NODES = NUM_USERS + NUM_ENTITIES
NNZ = 1_000_000
D = 64
H = 2
HD = 32
L = 50
B = 512
N_BLOCKS = 2
ALPHA = 0.2
NEG30 = -30000.0
EPS = 1e-12

NCORES = 8
SEG = 30208                  # int16 segment rows for 120832-row tables
ISEG = 25024                 # segment rows for item_emb (50048)
LP = 64
SEQ_PER_CORE = 64
NTILE = 32                   # token tiles per core (4096 tokens)
NTOK = NTILE * 128
IPC = NUM_ITEMS // NCORES    # 6250
IPAD = 6272                  # 49*128
NKI = IPAD // 128            # 49
R1 = N_NODES // NCORES       # 15000 rows/core for L1
NCH1 = (R1 + 127) // 128     # 118
SLOT1 = NCH1 * 128           # 15104
SCALE = float(1.0 / np.sqrt(np.float32(HD)))

F16 = mybir.dt.float16
F32 = mybir.dt.float32
I32 = mybir.dt.int32
I16 = mybir.dt.int16
AX = mybir.AxisListType
ALU = mybir.AluOpType
ACT = mybir.ActivationFunctionType


# =========================================================================
# host prep
# =========================================================================

def _csr(rows_sel, cols_sel, vals_sel, n_rows):
    """CSR over local row ids 0..n_rows-1."""
    deg = np.bincount(rows_sel, minlength=n_rows).astype(np.int64)
    order = np.argsort(rows_sel, kind="stable")
    rptr = np.zeros(n_rows + 1, np.int64)
    np.cumsum(deg, out=rptr[1:])
    return deg, cols_sel[order], vals_sel[order], rptr


def _chunk_schedule(deg, nch):
    """Degree-desc ordering of rows into nch chunks of 128 slots.
    Returns (order, K_list) with order padded to nch*128 using -1."""
    n = len(deg)
    order = np.argsort(-deg, kind="stable")
    pad = np.full(nch * 128 - n, -1, np.int64)
    order = np.concatenate([order, pad])
    K = [max(int(deg[order[c * 128]]), 1) if order[c * 128] >= 0 else 1
         for c in range(nch)]
    return order, K


def _seg_lists(order, deg, csr_cols, csr_vals, rptr, nch, seg, extra=None):
    """Per (chunk, segment): per-slot [cols_local, vals] lists.
    extra: optional per-row (col, val) appended edge (e.g. x1-self).
    Returns segK[ch][s] and lists[ch][s][p] = (locals, vals)."""
    segK = [[0] * 4 for _ in range(nch)]
    lists = [[[([], []) for _ in range(128)] for _ in range(4)]
             for _ in range(nch)]
    for ch in range(nch):
        sl = order[ch * 128:(ch + 1) * 128]
        for p, r in enumerate(sl):
            if r < 0:
                continue
            d0 = int(deg[r])
            cols = list(csr_cols[rptr[r]:rptr[r] + d0])
            vals = list(csr_vals[rptr[r]:rptr[r] + d0])
            if extra is not None:
                ec, ev = extra(r)
                cols += ec
                vals += ev
            for cc, vv in zip(cols, vals):
                s = int(cc) // seg
                lo, lv = lists[ch][s][p]
                lo.append(int(cc) % seg)
                lv.append(vv)
        for s in range(4):
            segK[ch][s] = max(
                (len(lists[ch][s][p][0]) for p in range(128)), default=0)
    return segK, lists


def _emit_sched(segK_all, lists_all, nch):
    """Common-K across cores; emit per-core (idx16 concat, val [128,S]).
    Returns (Kcom[ch][s], per-core idx16 [16, 8*S], per-core val [128, S])."""
    ncores = len(segK_all)
    Kcom = [[max(max(segK_all[c][ch][s], 1) if s == 0 else
                 segK_all[c][ch][s] for c in range(ncores))
             for s in range(4)] for ch in range(nch)]
    Stot = sum(sum(Kcom[ch]) for ch in range(nch))
    out = []
    for c in range(ncores):
        idx16 = np.zeros((128, 8 * Stot), np.int16)
        val = np.zeros((128, Stot), np.float32)
        off = 0
        for ch in range(nch):
            for s in range(4):
                Ks = Kcom[ch][s]
                if Ks == 0:
                    continue
                i2d = np.zeros((128, Ks), np.int32)
                for p in range(128):
                    lo, lv = lists_all[c][ch][s][p]
                    n0 = len(lo)
                    if n0:
                        i2d[p, :n0] = lo
                        val[p, off:off + n0] = lv
                flat = slots_to_flat(i2d, Ks)
                idx16[:, 8 * off:8 * (off + Ks)] = wrap16(flat)
                off += Ks
        out.append((idx16, val))
    return Kcom, Stot, out


def wrap16(flat_idx):
    """Flat gather order n -> int16 [128, ceil(n/16)]: n at [n%16, n//16],
    and the 16-partition block replicated to all 8 GPSIMD core stripes."""
    n = len(flat_idx)
    nc16 = (n + 15) // 16
    blk = np.zeros((16, nc16), np.int16)
    blk[np.arange(n) % 16, np.arange(n) // 16] = flat_idx.astype(np.int16)
    return np.tile(blk, (8, 1))


def slots_to_flat(idx2d, K):
    """[128, K] slot matrix -> flat order n = k*128 + p."""
    return idx2d[:, :K].T.reshape(-1)


def host_prep(inputs):
    f32 = np.float32
    seq = np.asarray(inputs["sequences"])
    arows = np.asarray(inputs["adj_rows"]).astype(np.int64)
    acols = np.asarray(inputs["adj_cols"]).astype(np.int64)
    avals = np.asarray(inputs["adj_vals"]).astype(f32)
    i2e = np.asarray(inputs["item_to_entity"]).astype(np.int64)

    P = {"shared": {}, "cores": [dict() for _ in range(NCORES)]}
    S = P["shared"]

    x0 = np.concatenate([np.asarray(inputs["user_emb_kg"]),
                         np.asarray(inputs["ent_emb_kg"])], 0).astype(f32)
    x0p = np.zeros((SEG * 4, D), f32)
    x0p[:N_NODES] = x0
    S["x0t"] = x0p
    iemb = np.zeros((ISEG * 2, D), f32)
    iemb[:NUM_ITEMS] = np.asarray(inputs["item_emb"]).astype(f32)
    S["iemb32"] = iemb

    # ---------------- L1 schedule (row shard) ----------------
    core_of_row = np.minimum(arows // R1, NCORES - 1)
    segK_all, lists_all = [], []
    inv1 = np.empty(N_NODES, np.int64)
    for c in range(NCORES):
        m = core_of_row == c
        r = arows[m] - c * R1
        deg, cc, vv, rptr = _csr(r, acols[m], avals[m], R1)
        order, _ = _chunk_schedule(deg, NCH1)
        sk, ls = _seg_lists(order, deg, cc, vv, rptr, NCH1, SEG)
        segK_all.append(sk)
        lists_all.append(ls)
        valid = order >= 0
        inv_local = np.empty(R1, np.int64)
        inv_local[order[valid]] = np.nonzero(valid)[0]
        inv1[c * R1:(c + 1) * R1] = c * SLOT1 + inv_local
    K1, S1, emitted = _emit_sched(segK_all, lists_all, NCH1)
    for c in range(NCORES):
        P["cores"][c]["ix1"] = emitted[c][0]
        P["cores"][c]["val1"] = emitted[c][1].astype(np.float16)
    S["S1"] = S1
    S["K1"] = K1

    # ---------------- L2 needed entities ----------------
    # per core: token entities (batch shard) + item entities (vocab shard)
    ent_rows_m = arows >= NUM_USERS
    er = arows[ent_rows_m] - NUM_USERS      # entity-local target
    ec = acols[ent_rows_m]
    ev = avals[ent_rows_m]
    # CSR over all entities once
    degE, ccE, vvE, rptrE = _csr(er, ec, ev, NUM_ENTITIES)

    nonpad = seq > 0
    idx0 = np.maximum(seq - 1, 0)
    ent_ids = i2e[idx0]                      # [B, L]
    tok_valid = (ent_ids >= 0) & nonpad
    seq_len = np.clip(nonpad.sum(1), 1, None)

    needed_all, nch2_all = [], []
    for c in range(NCORES):
        sl = slice(c * SEQ_PER_CORE, (c + 1) * SEQ_PER_CORE)
        te = ent_ids[sl][tok_valid[sl]]
        gid = c * IPC + np.arange(IPC)
        ie = i2e[gid]
        ie = ie[ie >= 0]
        needed = np.unique(np.concatenate([te, ie]))
        needed_all.append(needed)
        nch2_all.append((len(needed) + 127) // 128)
    NCH2 = ((max(nch2_all) + 15) // 16) * 16
    S["NCH2"] = NCH2
    SLOT2 = NCH2 * 128

    segK2_all, lists2_all, l2meta = [], [], []
    for c in range(NCORES):
        needed = needed_all[c]
        degN = degE[needed]
        order, _ = _chunk_schedule(degN, NCH2)

        def csr_cols_local(j, needed=needed, degN=degN):
            e = needed[j]
            d0 = int(degN[j])
            return e, d0

        # build per-row edge arrays in inv1 (X1TAB slot) space + x1 self
        deg_l = degN
        cc_l = np.zeros(int(degN.sum()), np.int64)
        vv_l = np.zeros(int(degN.sum()), np.float32)
        rp_l = np.zeros(len(needed) + 1, np.int64)
        np.cumsum(degN, out=rp_l[1:])
        for j, e in enumerate(needed):
            d0 = int(degN[j])
            if d0:
                cc_l[rp_l[j]:rp_l[j] + d0] = inv1[ccE[rptrE[e]:rptrE[e] + d0]]
                vv_l[rp_l[j]:rp_l[j] + d0] = \
                    vvE[rptrE[e]:rptrE[e] + d0] / 3.0

        def extra(j, needed=needed):
            return ([int(inv1[NUM_USERS + needed[j]])], [1.0 / 3.0])

        sk, ls = _seg_lists(order, deg_l, cc_l, vv_l, rp_l, NCH2, SEG,
                            extra=extra)
        segK2_all.append(sk)
        lists2_all.append(ls)
        sinv = np.zeros(NUM_ENTITIES, np.int64)
        valid = order >= 0
        sinv[needed[order[valid]]] = np.nonzero(valid)[0]
        # x0-self: per slot, x0 row id (node space) + seg masks
        x0row = np.zeros(NCH2 * 128, np.int64)
        x0row[np.nonzero(valid)[0]] = NUM_USERS + needed[order[valid]]
        l2meta.append((needed, sinv, x0row, valid.copy()))
    K2, S2, emitted2 = _emit_sched(segK2_all, lists2_all, NCH2)
    S["K2"] = K2
    S["S2"] = S2

    # x0-self pass: super-chunks of G2 chunks, 4 seg gathers each (full
    # slot layout), with per-seg val masks
    G2 = 16
    NSC = (NCH2 + G2 - 1) // G2
    S["NSC"] = NSC
    S["G2"] = G2
    for c in range(NCORES):
        C = P["cores"][c]
        C["ix2"], C["val2"] = emitted2[c][0], emitted2[c][1].astype(
            np.float16)
        needed, sinv, x0row, slot_valid = l2meta[c]
        ixs = np.zeros((128, NSC * 4 * 8 * G2), np.int16)
        msk = np.zeros((128, NSC * 4 * G2), np.float16)
        col = 0
        for sc in range(NSC):
            ch0 = sc * G2
            slots = np.arange(ch0 * 128, min((ch0 + G2) * 128, NCH2 * 128))
            rows = x0row[slots]
            vmask = slot_valid[slots]
            G = G2
            # pad slots to G2*128
            if len(slots) < G2 * 128:
                pad = G2 * 128 - len(slots)
                rows = np.concatenate([rows, np.zeros(pad, np.int64)])
                vmask = np.concatenate([vmask, np.zeros(pad, bool)])
            for s in range(4):
                inseg = (rows // SEG) == s
                loc = np.where(inseg, rows % SEG, 0)
                i2d = loc.reshape(G, 128).T  # [128, G] (slot = ch*128+p)
                flat = slots_to_flat(i2d.astype(np.int32), G)
                ixs[:, 8 * col:8 * (col + G)] = wrap16(flat)
                mm2 = (inseg & vmask).reshape(G, 128).T.astype(np.float16)
                msk[:, col:col + G] = mm2 / 3.0
                col += G
        C["ix0s"] = ixs
        C["msk0"] = msk
        l2meta[c] = (needed, sinv)

    # ---------------- token path ----------------
    tril = np.tril(np.ones((LP, LP), bool))
    for c in range(NCORES):
        C = P["cores"][c]
        needed, sinv = l2meta[c]
        sl = slice(c * SEQ_PER_CORE, (c + 1) * SEQ_PER_CORE)
        sq = seq[sl]
        np_m = nonpad[sl]
        va_m = tok_valid[sl]
        i0 = idx0[sl]
        eids = np.clip(ent_ids[sl], 0, NUM_ENTITIES - 1)

        tok_idx = np.zeros((128, NTILE), np.int64)
        ent_sidx = np.zeros((128, NTILE), np.int32)
        npm = np.zeros((128, NTILE), np.float32)
        vam = np.zeros((128, NTILE), np.float16)
        for n in range(NTOK):
            p, k = n % 128, n // 128
            b_loc, l = n // LP, n % LP
            if l < L:
                tok_idx[p, k] = i0[b_loc, l]
                ent_sidx[p, k] = sinv[eids[b_loc, l]]
                npm[p, k] = np_m[b_loc, l]
                vam[p, k] = va_m[b_loc, l]
        # item-emb gather split into 2 segments, full slot layout each
        ix_e = np.zeros((128, 2 * 8 * NTILE), np.int16)
        npm_s = np.zeros((128, 2 * NTILE), np.float16)
        for s in range(2):
            inseg = (tok_idx // ISEG) == s
            loc = np.where(inseg, tok_idx % ISEG, 0)
            flat = slots_to_flat(loc.astype(np.int32), NTILE)
            ix_e[:, s * 8 * NTILE:(s + 1) * 8 * NTILE] = wrap16(flat)
            npm_s[:, s * NTILE:(s + 1) * NTILE] = \
                (inseg.astype(np.float32) * npm).astype(np.float16)
        C["ix_emb"] = ix_e
        C["npm_s"] = npm_s
        C["npm"] = npm.astype(np.float16)
        C["ix_sg"] = wrap16(slots_to_flat(ent_sidx, NTILE))
        C["vam"] = vam

        mm = np.zeros((128, NTILE * 128), np.float16)
        ma = np.zeros((128, NTILE * 128), np.float32)
        for t in range(NTILE):
            for half in range(2):
                b_loc = t * 2 + half
                va = np.zeros(LP, bool)
                va[:L] = va_m[b_loc]
                sp = np.ones(LP, bool)
                sp[:L] = sq[b_loc] == 0
                ps = half * 64
                for h in range(H):
                    fs = t * 128 + h * 64
                    vv = va[:, None] & va[None, :]
                    kg_m = np.where(vv, ALPHA * SCALE, 0.0)
                    kg_a = np.where(vv, 0.0, ALPHA * NEG30)
                    causal = np.where(tril, 0.0, NEG30)
                    add = causal + kg_a + np.where(sp[None, :], NEG30, 0.0)
                    mm[ps:ps + 64, fs:fs + 64] = kg_m
                    ma[ps:ps + 64, fs:fs + 64] = np.maximum(add, NEG30)
        C["MM"] = mm
        C["MA"] = np.maximum(ma, NEG30).astype(np.float16)

        lt = np.zeros(128, np.int32)
        for b_loc in range(SEQ_PER_CORE):
            n = b_loc * LP + int(seq_len[sl][b_loc]) - 1
            p, k = n % 128, n // 128
            lt[b_loc] = p * NTILE + k
        C["ix_uv"] = wrap16(lt)

        # item path
        j = np.arange(IPAD)
        gid = c * IPC + np.minimum(j, IPC - 1)
        inr = j < IPC
        ie = np.where(inr, i2e[gid], -1)
        va_i = (ie >= 0) & inr
        eclip = np.clip(ie, 0, NUM_ENTITIES - 1)
        isx = np.zeros((128, NKI), np.int32)
        iva = np.zeros((128, NKI), np.float16)
        isx[j % 128, j // 128] = sinv[eclip]
        iva[j % 128, j // 128] = va_i
        C["ix_si"] = wrap16(slots_to_flat(isx, NKI))
        C["item_va"] = iva
        embT = np.zeros((64, IPAD), np.float16)
        embT[:, :IPC] = np.asarray(inputs["item_emb"])[c * IPC:(c + 1) * IPC].T
        C["iembT"] = embT

    # ---------------- weights ----------------
    fuse_W = np.asarray(inputs["fuse_W"]).astype(f32)
    W_ie = np.ascontiguousarray(fuse_W[:D])
    W_se = (np.asarray(inputs["kg2e_item_W"]) @ fuse_W[D:]).astype(f32)
    b_f = (np.asarray(inputs["fuse_b"])
           + np.asarray(inputs["kg2e_item_b"]) @ fuse_W[D:]).astype(f32)

    w64 = []
    for i in range(N_BLOCKS):
        w64.append(np.asarray(inputs["blk_Wq"])[i] * SCALE)
        w64.append(np.asarray(inputs["blk_Wk"])[i])
        w64.append(np.asarray(inputs["blk_Wv"])[i])
        w64.append(np.asarray(inputs["blk_Wo"])[i])
    w64 += [np.asarray(inputs["kg2e_tok_W"]), np.asarray(inputs["kgq_W"]),
            np.asarray(inputs["kgk_W"]), W_ie, W_se]
    S["W64"] = np.concatenate([w.astype(np.float16) for w in w64],
                              axis=1)  # [64, 13*64]
    S["W1s"] = np.concatenate([np.asarray(inputs["blk_W1"])[i].astype(
        np.float16) for i in range(N_BLOCKS)], axis=1)      # [64, 512]
    w2 = []
    for i in range(N_BLOCKS):
        W2 = np.asarray(inputs["blk_W2"])[i]                # [256, 64]
        w2.append(np.concatenate([W2[:128], W2[128:]], axis=1))  # [128,128]
    S["W2s"] = np.concatenate(w2, axis=1).astype(np.float16)  # [128, 256]

    # replicated [128, 64] f32 constants
    rep = lambda v: np.broadcast_to(np.asarray(v, f32)[None, :],
                                    (128, D)).copy()
    c64 = [rep(inputs["ln_g"]), rep(inputs["ln_b"])]
    # pos_rep: partition p -> pos_emb[p % 64] (l = p % 64), zero for l >= L
    pe = np.zeros((LP, D), f32)
    pe[:L] = np.asarray(inputs["pos_emb"])
    c64.append(np.concatenate([pe, pe], axis=0))
    for i in range(N_BLOCKS):
        c64 += [rep(np.asarray(inputs["blk_bv"])[i]),
                rep(np.asarray(inputs["blk_bo"])[i]),
                rep(np.asarray(inputs["blk_ln1_g"])[i]),
                rep(np.asarray(inputs["blk_ln1_b"])[i]),
                rep(np.asarray(inputs["blk_b2"])[i]),
                rep(np.asarray(inputs["blk_ln2_g"])[i]),
                rep(np.asarray(inputs["blk_ln2_b"])[i])]
    S["C64"] = np.concatenate(c64, axis=1)  # [128, (3+14)*64]
    S["C256"] = np.concatenate(
        [np.broadcast_to(np.asarray(inputs["blk_b1"])[i][None, :],
                         (128, 4 * D)).astype(f32).copy()
         for i in range(N_BLOCKS)], axis=1)  # [128, 512]
    # per-partition biases on 64 partitions: b_tok, b_f, bq0,bk0,bq1,bk1
    cp = [np.asarray(inputs["kg2e_tok_b"]), b_f]
    for i in range(N_BLOCKS):
        cp += [np.asarray(inputs["blk_bq"])[i], np.asarray(inputs["blk_bk"])[i]]
    S["CP"] = np.stack(cp, axis=1).astype(f32)  # [64, 6]
    return P


# =========================================================================
# device kernel builder
# =========================================================================

def build_nc(S, lvl=5):
    S1, S2, K1, K2, NCH2 = S["S1"], S["S2"], S["K1"], S["K2"], S["NCH2"]
    NSC, G2 = S["NSC"], S["G2"]
    SLOT2 = NCH2 * 128
    nc = bass.Bass()
    dt = nc.dram_tensor
    # inputs
    x0t = dt("x0t", [SEG * 4, D], F32, kind="ExternalInput")
    iemb32 = dt("iemb32", [ISEG * 2, D], F32, kind="ExternalInput")
    ix1_in = dt("ix1", [128, 8 * S1], I16, kind="ExternalInput")
    val1 = dt("val1", [128, S1], F16, kind="ExternalInput")
    ix2_in = dt("ix2", [128, 8 * S2], I16, kind="ExternalInput")
    val2 = dt("val2", [128, S2], F16, kind="ExternalInput")
    ix0s_in = dt("ix0s", [128, NSC * 4 * 8 * G2], I16, kind="ExternalInput")
    msk0_in = dt("msk0", [128, NSC * 4 * G2], F16, kind="ExternalInput")
    ixemb_in = dt("ix_emb", [128, 2 * 8 * NTILE], I16, kind="ExternalInput")
    npms_in = dt("npm_s", [128, 2 * NTILE], F16, kind="ExternalInput")
    ixsg_in = dt("ix_sg", [128, 8 * NTILE], I16, kind="ExternalInput")
    ixsi_in = dt("ix_si", [128, 8 * NKI], I16, kind="ExternalInput")
    ixuv_in = dt("ix_uv", [128, 8], I16, kind="ExternalInput")
    npm_in = dt("npm", [128, NTILE], F16, kind="ExternalInput")
    vam_in = dt("vam", [128, NTILE], F16, kind="ExternalInput")
    MM_in = dt("MM", [128, NTILE * 128], F16, kind="ExternalInput")
    MA_in = dt("MA", [128, NTILE * 128], F16, kind="ExternalInput")
    item_va = dt("item_va", [128, NKI], F16, kind="ExternalInput")
    iembT_in = dt("iembT", [64, IPAD], F16, kind="ExternalInput")
    W64_in = dt("W64", [64, 13 * 64], F16, kind="ExternalInput")
    W1s_in = dt("W1s", [64, 512], F16, kind="ExternalInput")
    W2s_in = dt("W2s", [128, 256], F16, kind="ExternalInput")
    C64_in = dt("C64", [128, 17 * 64], F32, kind="ExternalInput")
    C256_in = dt("C256", [128, 512], F32, kind="ExternalInput")
    CP_in = dt("CP", [64, 6], F32, kind="ExternalInput")
    out = dt("out", [B, IPAD], F32, kind="ExternalOutput")

    with tile.TileContext(nc) as tc, contextlib.ExitStack() as ES:
        dram = ES.enter_context(tc.tile_pool(name="dram", bufs=1,
                                             space="DRAM"))
        x1b = dram.tile([SLOT1, D], F32)
        X1TAB = dram.tile([NCORES * SLOT1, D], F32, addr_space="Shared")
        SDRAM = dram.tile([SLOT2, D], F32)
        hD = dram.tile([NTOK, D], F32)
        uvb = dram.tile([SEQ_PER_CORE, D], F32)
        UV = dram.tile([B, D], F32, addr_space="Shared")

        cst = ES.enter_context(tc.tile_pool(name="cst", bufs=1))
        # constants
        ixemb_t = cst.tile([128, 2 * 8 * NTILE], I16)
        npms_t = cst.tile([128, 2 * NTILE], F16)
        ixsg_t = cst.tile([128, 8 * NTILE], I16)
        ixsi_t = cst.tile([128, 8 * NKI], I16)
        ixuv_t = cst.tile([128, 8], I16)
        nc.sync.dma_start(ixemb_t[:], ixemb_in[:])
        nc.sync.dma_start(npms_t[:], npms_in[:])
        nc.sync.dma_start(ixsg_t[:], ixsg_in[:])
        nc.sync.dma_start(ixsi_t[:], ixsi_in[:])
        nc.sync.dma_start(ixuv_t[:], ixuv_in[:])
        npm_t = cst.tile([128, NTILE], F16)
        vam_t = cst.tile([128, NTILE], F16)
        nc.sync.dma_start(npm_t[:], npm_in[:])
        nc.sync.dma_start(vam_t[:], vam_in[:])
        MMt = cst.tile([128, NTILE * 128], F16)
        MAt = cst.tile([128, NTILE * 128], F16)
        nc.sync.dma_start(MMt[:], MM_in[:])
        nc.sync.dma_start(MAt[:], MA_in[:])
        iva_t = cst.tile([128, NKI], F16)
        nc.sync.dma_start(iva_t[:], item_va[:])
        iembT = cst.tile([64, IPAD], F16)
        nc.sync.dma_start(iembT[:], iembT_in[:])
        W64 = cst.tile([64, 13 * 64], F16)
        W1s = cst.tile([64, 512], F16)
        W2s = cst.tile([128, 256], F16)
        nc.sync.dma_start(W64[:], W64_in[:])
        nc.sync.dma_start(W1s[:], W1s_in[:])
        nc.sync.dma_start(W2s[:], W2s_in[:])
        C64 = cst.tile([128, 17 * 64], F32)
        C256 = cst.tile([128, 512], F32)
        CP = cst.tile([64, 6], F32)
        nc.sync.dma_start(C64[:], C64_in[:])
        nc.sync.dma_start(C256[:], C256_in[:])
        nc.sync.dma_start(CP[:], CP_in[:])
        ident = cst.tile([128, 128], F16)
        make_identity(nc, ident[:])
        nc.gpsimd.load_library(library_config.mlp)
        eps_t = cst.tile([128, 1], F32)
        nc.gpsimd.memset(eps_t[:], EPS)

        def W(i):  # [64, 64] f16 slice of W64
            return W64[:, i * 64:(i + 1) * 64]
        WqS = [W(0), W(4)]
        Wk = [W(1), W(5)]
        Wv = [W(2), W(6)]
        Wo = [W(3), W(7)]
        W_tok, Wkgq, Wkgk, W_ie, W_se = W(8), W(9), W(10), W(11), W(12)

        def C(i):
            return C64[:, i * 64:(i + 1) * 64]
        ln_g, ln_b, pos_rep = C(0), C(1), C(2)
        bv_r = [C(3), C(10)]
        bo_r = [C(4), C(11)]
        ln1g = [C(5), C(12)]
        ln1b = [C(6), C(13)]
        b2_r = [C(7), C(14)]
        ln2g = [C(8), C(15)]
        ln2b = [C(9), C(16)]
        b_tokT = CP[:, 0:1]
        b_fT = CP[:, 1:2]
        bqT = [CP[:, 2:3], CP[:, 4:5]]
        bkT = [CP[:, 3:4], CP[:, 5:6]]

        _snapc = {}

        def sreg(v):
            if v not in _snapc:
                _snapc[v] = nc.gpsimd.to_reg(v)
            return _snapc[v]

        def seg_gather_chunk(g, Ks, ix_t, off, table, seg_rows):
            """Fill g [128, sum(Ks)*64] via per-segment dma_gather calls."""
            co = 0
            for s2 in range(4):
                Ksg = Ks[s2]
                if Ksg == 0:
                    continue
                nc.gpsimd.dma_gather(
                    out_ap=g[:, co * 64:(co + Ksg) * 64].rearrange(
                        "p (k d) -> p k d", d=64),
                    in_ap=table[s2 * seg_rows:(s2 + 1) * seg_rows, :],
                    idxs_ap=ix_t[:, 8 * (off + co):8 * (off + co + Ksg)],
                    num_idxs=128 * Ksg,
                    num_idxs_reg=sreg(128 * Ksg),
                    elem_size=64, single_packet=False)
                co += Ksg

        # ---------------- GCN L1 ----------------
        with tc.tile_pool(name="x1p", bufs=1) as x1p, \
             tc.tile_pool(name="c1", bufs=1) as c1p, \
             tc.tile_pool(name="g1", bufs=2) as g1p, \
             tc.tile_pool(name="m1", bufs=2) as m1p:
            ix1_t = c1p.tile([128, 8 * S1], I16)
            val1_t = c1p.tile([128, S1], F16)
            nc.sync.dma_start(ix1_t[:], ix1_in[:])
            nc.sync.dma_start(val1_t[:], val1[:])
            x1res = x1p.tile([128, NCH1 * 64], F32)
            off = 0
            for ch in range(NCH1):
                K = sum(K1[ch])
                g = g1p.tile([128, K * 64], F32, tag="g")
                seg_gather_chunk(g, K1[ch], ix1_t, off, x0t, SEG)
                m = m1p.tile([128, K * 64], F16, tag="m")
                nc.vector.tensor_tensor(
                    out=m[:],
                    in0=g[:].rearrange("p (k d) -> p k d", d=64),
                    in1=val1_t[:, off:off + K].rearrange(
                        "p k -> p k ()").to_broadcast([128, K, 64]),
                    op=ALU.mult)
                nc.vector.tensor_reduce(
                    out=x1res[:, ch * 64:(ch + 1) * 64],
                    in_=m[:].rearrange("p (k d) -> p d k", d=64),
                    axis=AX.X, op=ALU.add)
                off += K
            # store shard, then AllGather
            nc.sync.dma_start(
                x1b[:].rearrange("(c p) d -> p c d", p=128),
                x1res[:].rearrange("p (c d) -> p c d", d=64))
            nc.gpsimd.collective_compute(
                "AllGather", ALU.bypass,
                replica_groups=[list(range(NCORES))],
                ins=[x1b[:]], outs=[X1TAB[:]])
            if lvl == 1:
                nc.gpsimd.dma_start(out=out[0:128, 0:IPAD],
                                    in_=x1res[:, 0:IPAD])

        # ---------------- GCN L2 (s table) ----------------
        if lvl < 2:
            return nc
        with tc.tile_pool(name="sp", bufs=1) as sp, \
             tc.tile_pool(name="c2", bufs=1) as c2p, \
             tc.tile_pool(name="g2", bufs=2) as g2p, \
             tc.tile_pool(name="m2", bufs=2) as m2p:
            ix2_t = c2p.tile([128, 8 * S2], I16)
            val2_t = c2p.tile([128, S2], F16)
            ix0s_t = c2p.tile([128, NSC * 4 * 8 * G2], I16)
            msk0_t = c2p.tile([128, NSC * 4 * G2], F16)
            nc.sync.dma_start(ix2_t[:], ix2_in[:])
            nc.sync.dma_start(val2_t[:], val2[:])
            nc.sync.dma_start(ix0s_t[:], ix0s_in[:])
            nc.sync.dma_start(msk0_t[:], msk0_in[:])
            sres = sp.tile([128, NCH2 * 64], F32)
            off = 0
            for ch in range(NCH2):
                K = sum(K2[ch])
                g = g2p.tile([128, K * 64], F32, tag="g2")
                seg_gather_chunk(g, K2[ch], ix2_t, off, X1TAB[:], SEG)
                m = m2p.tile([128, K * 64], F16, tag="m2")
                nc.vector.tensor_tensor(
                    out=m[:],
                    in0=g[:].rearrange("p (k d) -> p k d", d=64),
                    in1=val2_t[:, off:off + K].rearrange(
                        "p k -> p k ()").to_broadcast([128, K, 64]),
                    op=ALU.mult)
                nc.vector.tensor_reduce(
                    out=sres[:, ch * 64:(ch + 1) * 64],
                    in_=m[:].rearrange("p (k d) -> p d k", d=64),
                    axis=AX.X, op=ALU.add)
                off += K
            # x0-self pass: add x0/3 into sres per super-chunk
            col = 0
            for sc in range(NSC):
                for s2 in range(4):
                    g0 = g2p.tile([128, G2 * 64], F32, tag="g0s")
                    nc.gpsimd.dma_gather(
                        out_ap=g0[:].rearrange("p (k d) -> p k d", d=64),
                        in_ap=x0t[s2 * SEG:(s2 + 1) * SEG, :],
                        idxs_ap=ix0s_t[:,
                                       (sc * 4 + s2) * 8 * G2:
                                       (sc * 4 + s2 + 1) * 8 * G2],
                        num_idxs=128 * G2,
                        num_idxs_reg=sreg(128 * G2),
                        elem_size=64, single_packet=False)
                    m0 = m2p.tile([128, G2 * 64], F32, tag="m0s")
                    mc = (sc * 4 + s2) * G2
                    nc.vector.tensor_tensor(
                        out=m0[:].rearrange("p (k d) -> p k d", d=64),
                        in0=g0[:].rearrange("p (k d) -> p k d", d=64),
                        in1=msk0_t[:, mc:mc + G2].rearrange(
                            "p k -> p k ()").to_broadcast([128, G2, 64]),
                        op=ALU.mult)
                    c0 = sc * G2 * 64
                    nc.vector.tensor_tensor(
                        out=sres[:, c0:c0 + G2 * 64],
                        in0=sres[:, c0:c0 + G2 * 64],
                        in1=m0[:], op=ALU.add)
            nc.sync.dma_start(
                SDRAM[:].rearrange("(c p) d -> p c d", p=128),
                sres[:].rearrange("p (c d) -> p c d", d=64))
            if lvl == 2:
                nc.gpsimd.dma_start(out=out[0:128, 0:NCH2 * 64],
                                    in_=sres[:, 0:NCH2 * 64])

        # ---------------- token path: h0, kg bias ----------------
        if lvl < 3:
            return nc
        big = ES.enter_context(tc.tile_pool(name="big", bufs=1))
        h = big.tile([128, NTILE * 64], F32)      # residual stream
        h16 = big.tile([128, NTILE * 64], F16)    # f16 copy for PE
        MASK = big.tile([128, NTILE * 128], F16)  # attn mask (pre-softmax add)
        HT = big.tile([64, NTILE * 128], F16)     # h transposed
        ATT = big.tile([128, NTILE * 128], F16)
        SC = big.tile([128, NTILE * 128], F16)
        QT = big.tile([64, NTILE * 128], F16)
        KT = big.tile([64, NTILE * 128], F16)
        VV = big.tile([128, NTILE * 64], F16)
        F1 = big.tile([128, NTILE * 256], F16)
        IVT = big.tile([64, IPAD], F16)

        ps_t = ES.enter_context(tc.tile_pool(name="ps_t", bufs=2,
                                              space="PSUM"))
        ps_b = ES.enter_context(tc.tile_pool(name="ps_b", bufs=2,
                                             space="PSUM"))
        ps_s = ES.enter_context(tc.tile_pool(name="ps_s", bufs=2,
                                             space="PSUM"))
        tmp = ES.enter_context(tc.tile_pool(name="tmp", bufs=3))
        lnp = ES.enter_context(tc.tile_pool(name="lnp", bufs=1))

        def transpose_to(dst_ap, src_ap, w=128):
            """PE transpose [128, w] -> psum [w, 128] -> copy to dst."""
            pt = ps_t.tile([128, 128], F16, tag="ptT")
            nc.tensor.transpose(out=pt[:w, :src_ap.shape[0]], in_=src_ap,
                                identity=ident[:])
            nc.scalar.activation(dst_ap, pt[:w, :src_ap.shape[0]], ACT.Copy)

        def ln_inplace(x_ap, g_ap, b_ap, nt=NTILE):
            """LayerNorm over last-64 groups of x [128, nt*64] f32."""
            mu = tmp.tile([128, nt], F32, tag="mu")
            var = tmp.tile([128, nt], F32, tag="var")
            xc = lnp.tile([128, nt * 64], F32, tag="xc")
            x3 = x_ap.rearrange("p (k d) -> p k d", d=64)
            nc.vector.tensor_reduce(out=mu[:], in_=x3, axis=AX.X, op=ALU.add,
                                    negate=True)
            nc.scalar.activation(mu[:], mu[:], ACT.Copy, scale=1.0 / 64)
            nc.vector.tensor_tensor(
                out=xc[:], in0=x3,
                in1=mu[:].rearrange("p k -> p k ()").to_broadcast([128, nt, 64]),
                op=ALU.add)
            xc3 = xc[:].rearrange("p (k d) -> p k d", d=64)
            sq = lnp.tile([128, nt * 64], F32, tag="sq")
            nc.vector.tensor_tensor(out=sq[:], in0=xc3, in1=xc3, op=ALU.mult)
            nc.vector.tensor_reduce(out=var[:],
                                    in_=sq[:].rearrange("p (k d) -> p k d",
                                                        d=64),
                                    axis=AX.X, op=ALU.add)
            nc.scalar.activation(var[:], var[:], ACT.Sqrt, bias=eps_t[:],
                                 scale=1.0 / 64)
            nc.vector.reciprocal(var[:], var[:])
            # var now = 1/sqrt(mean(xc^2)+eps) * ... note scale folds 1/64
            nc.vector.tensor_tensor(
                out=xc[:], in0=xc3,
                in1=var[:].rearrange("p k -> p k ()").to_broadcast(
                    [128, nt, 64]),
                op=ALU.mult)
            nc.vector.tensor_tensor(
                out=xc[:], in0=xc3,
                in1=g_ap.rearrange("p d -> p () d").to_broadcast([128, nt, 64]),
                op=ALU.mult)
            nc.vector.tensor_tensor(
                out=x_ap, in0=xc3,
                in1=b_ap.rearrange("p d -> p () d").to_broadcast([128, nt, 64]),
                op=ALU.add)

        # h0 = LN(e0*npm_s0 + e1*npm_s1 + pos)
        with tc.tile_pool(name="h0p", bufs=1) as h0p:
            h3 = h[:].rearrange("p (k d) -> p k d", d=64)
            for s2 in range(2):
                emb = h0p.tile([128, NTILE * 64], F32, tag="emb")
                nc.gpsimd.dma_gather(
                    out_ap=emb[:].rearrange("p (k d) -> p k d", d=64),
                    in_ap=iemb32[s2 * ISEG:(s2 + 1) * ISEG, :],
                    idxs_ap=ixemb_t[:, s2 * 8 * NTILE:(s2 + 1) * 8 * NTILE],
                    num_idxs=NTOK, num_idxs_reg=sreg(NTOK), elem_size=64, single_packet=False)
                em = h0p.tile([128, NTILE * 64], F32, tag="em")
                nc.vector.tensor_tensor(
                    out=em[:].rearrange("p (k d) -> p k d", d=64),
                    in0=emb[:].rearrange("p (k d) -> p k d", d=64),
                    in1=npms_t[:, s2 * NTILE:(s2 + 1) * NTILE].rearrange(
                        "p k -> p k ()").to_broadcast([128, NTILE, 64]),
                    op=ALU.mult)
                if s2 == 0:
                    nc.vector.tensor_copy(h[:], em[:])
                else:
                    nc.vector.tensor_tensor(out=h[:], in0=h[:], in1=em[:],
                                            op=ALU.add)
            nc.vector.tensor_tensor(
                out=h3, in0=h3,
                in1=pos_rep.rearrange("p d -> p () d").to_broadcast(
                    [128, NTILE, 64]),
                op=ALU.add)
            ln_inplace(h[:], ln_g, ln_b)

        if lvl == 31:
            nc.gpsimd.dma_start(out=out[0:128, 0:NTILE * 64], in_=h[:])
            return nc
        # kg bias -> MASK
        with tc.tile_pool(name="kgp", bufs=1) as kgp:
            sg32 = kgp.tile([128, NTILE * 64], F32)
            nc.gpsimd.dma_gather(
                out_ap=sg32[:].rearrange("p (k d) -> p k d", d=64),
                in_ap=SDRAM[:],
                idxs_ap=ixsg_t[:, :],
                num_idxs=NTOK, num_idxs_reg=sreg(NTOK), elem_size=64, single_packet=False)
            sg = kgp.tile([128, NTILE * 64], F16)
            nc.vector.tensor_tensor(
                out=sg[:].rearrange("p (k d) -> p k d", d=64),
                in0=sg32[:].rearrange("p (k d) -> p k d", d=64),
                in1=vam_t[:].rearrange("p k -> p k ()").to_broadcast(
                    [128, NTILE, 64]),
                op=ALU.mult)
            sgT = kgp.tile([64, NTILE * 128], F16)
            for k in range(NTILE):
                transpose_to(sgT[:, k * 128:(k + 1) * 128],
                             sg[:, k * 64:(k + 1) * 64], w=64)
            if lvl == 315:
                nc.gpsimd.dma_start(out=out[0:64, 0:NTILE * 128], in_=sgT[:])
                return nc
            entT = kgp.tile([64, NTILE * 128], F16)
            for i in range(8):
                pb = ps_b.tile([64, 512], F32, tag="pb")
                nc.tensor.matmul(out=pb[:], lhsT=W_tok,
                                 rhs=sgT[:, i * 512:(i + 1) * 512],
                                 start=True, stop=True)
                nc.vector.tensor_scalar_add(
                    entT[:, i * 512:(i + 1) * 512], pb[:], b_tokT)
            if lvl == 316:
                nc.gpsimd.dma_start(out=out[0:64, 0:NTILE * 128],
                                    in_=entT[:])
                return nc
            for i in range(8):
                pb = ps_b.tile([64, 512], F32, tag="pb")
                nc.tensor.matmul(out=pb[:], lhsT=Wkgq,
                                 rhs=entT[:, i * 512:(i + 1) * 512],
                                 start=True, stop=True)
                nc.scalar.activation(QT[:, i * 512:(i + 1) * 512], pb[:],
                                     ACT.Copy)
            for i in range(8):
                pb = ps_b.tile([64, 512], F32, tag="pb")
                nc.tensor.matmul(out=pb[:], lhsT=Wkgk,
                                 rhs=entT[:, i * 512:(i + 1) * 512],
                                 start=True, stop=True)
                nc.scalar.activation(KT[:, i * 512:(i + 1) * 512], pb[:],
                                     ACT.Copy)
            if lvl == 317:
                nc.gpsimd.dma_start(out=out[0:64, 0:NTILE * 128], in_=QT[:])
                return nc
            for t in range(NTILE):
                for hh in range(2):
                    pk = ps_s.tile([128, 64], F32, tag="pk")
                    for s in range(2):
                        qs = QT[hh * 32:hh * 32 + 32,
                                t * 128 + s * 64:t * 128 + s * 64 + 64]
                        ks = KT[hh * 32:hh * 32 + 32,
                                t * 128 + s * 64:t * 128 + s * 64 + 64]
                        nc.tensor.matmul(
                            out=pk[s * 64:s * 64 + 64, :],
                            lhsT=qs, rhs=ks, start=True, stop=True)
                    c0 = t * 128 + hh * 64
                    nc.vector.tensor_tensor(
                        out=MASK[:, c0:c0 + 64], in0=pk[:],
                        in1=MMt[:, c0:c0 + 64], op=ALU.mult)
            nc.vector.tensor_tensor(out=MASK[:], in0=MASK[:], in1=MAt[:],
                                    op=ALU.add)

        if lvl == 32:
            nc.gpsimd.dma_start(out=out[0:128, 0:NTILE * 128], in_=MASK[:])
            return nc
        # item path -> IVT
        with tc.tile_pool(name="ivp", bufs=1) as ivp:
            sgi32 = ivp.tile([128, NKI * 64], F32)
            nc.gpsimd.dma_gather(
                out_ap=sgi32[:].rearrange("p (k d) -> p k d", d=64),
                in_ap=SDRAM[:],
                idxs_ap=ixsi_t[:, :],
                num_idxs=IPAD, num_idxs_reg=sreg(IPAD), elem_size=64, single_packet=False)
            sgi = ivp.tile([128, NKI * 64], F16)
            nc.vector.tensor_tensor(
                out=sgi[:].rearrange("p (k d) -> p k d", d=64),
                in0=sgi32[:].rearrange("p (k d) -> p k d", d=64),
                in1=iva_t[:].rearrange("p k -> p k ()").to_broadcast(
                    [128, NKI, 64]),
                op=ALU.mult)
            sgiT = ivp.tile([64, IPAD], F16)
            for k in range(NKI):
                transpose_to(sgiT[:, k * 128:(k + 1) * 128],
                             sgi[:, k * 64:(k + 1) * 64], w=64)
            for i in range(13):
                n0 = i * 512
                n1 = min(n0 + 512, IPAD)
                pb = ps_b.tile([64, 512], F32, tag="pb")
                nc.tensor.matmul(out=pb[:, :n1 - n0], lhsT=W_se,
                                 rhs=sgiT[:, n0:n1], start=True, stop=False)
                nc.tensor.matmul(out=pb[:, :n1 - n0], lhsT=W_ie,
                                 rhs=iembT[:, n0:n1], start=False, stop=True)
                nc.vector.tensor_scalar_add(IVT[:, n0:n1], pb[:, :n1 - n0],
                                            b_fT)

        if lvl == 3:
            nc.gpsimd.dma_start(out=out[0:128, 0:NTILE * 128],
                                in_=MASK[:])
            nc.gpsimd.dma_start(out=out[128:256, 0:NTILE * 64], in_=h[:])
            nc.gpsimd.dma_start(out=out[256:320, 0:IPAD], in_=IVT[:])
            return nc
        # ---------------- transformer blocks ----------------
        for i in range(N_BLOCKS):
            # hT
            nc.vector.tensor_copy(h16[:], h[:])
            for k in range(NTILE):
                transpose_to(HT[:, k * 128:(k + 1) * 128],
                             h16[:, k * 64:(k + 1) * 64], w=64)
            # qT, kT
            for j in range(8):
                pb = ps_b.tile([64, 512], F32, tag="pb")
                nc.tensor.matmul(out=pb[:], lhsT=WqS[i],
                                 rhs=HT[:, j * 512:(j + 1) * 512],
                                 start=True, stop=True)
                nc.vector.tensor_scalar_add(QT[:, j * 512:(j + 1) * 512],
                                            pb[:], bqT[i])
            for j in range(8):
                pb = ps_b.tile([64, 512], F32, tag="pb")
                nc.tensor.matmul(out=pb[:], lhsT=Wk[i],
                                 rhs=HT[:, j * 512:(j + 1) * 512],
                                 start=True, stop=True)
                nc.vector.tensor_scalar_add(KT[:, j * 512:(j + 1) * 512],
                                            pb[:], bkT[i])
            # v (token layout) + bias
            for k in range(NTILE):
                pv = ps_t.tile([128, 128], F32, tag="pt")
                nc.tensor.matmul(out=pv[:, :64],
                                 lhsT=HT[:, k * 128:(k + 1) * 128],
                                 rhs=Wv[i], start=True, stop=True)
                nc.vector.tensor_tensor(out=VV[:, k * 64:(k + 1) * 64],
                                        in0=pv[:, :64], in1=bv_r[i],
                                        op=ALU.add)
            # scores + softmax
            for t in range(NTILE):
                for hh in range(2):
                    pk = ps_s.tile([128, 64], F32, tag="pk")
                    for s in range(2):
                        qs = QT[hh * 32:hh * 32 + 32,
                                t * 128 + s * 64:t * 128 + s * 64 + 64]
                        ks = KT[hh * 32:hh * 32 + 32,
                                t * 128 + s * 64:t * 128 + s * 64 + 64]
                        nc.tensor.matmul(
                            out=pk[s * 64:s * 64 + 64, :],
                            lhsT=qs, rhs=ks, start=True, stop=True)
                    c0 = t * 128 + hh * 64
                    nc.vector.tensor_tensor(out=SC[:, c0:c0 + 64],
                                            in0=pk[:],
                                            in1=MASK[:, c0:c0 + 64],
                                            op=ALU.add)
            mx = tmp.tile([128, NTILE * 2], F32, tag="mx")
            sm = tmp.tile([128, NTILE * 2], F32, tag="sm")
            sc3 = SC[:].rearrange("p (g d) -> p g d", d=64)
            nc.vector.tensor_reduce(out=mx[:], in_=sc3, axis=AX.X, op=ALU.max,
                                    negate=True)
            nc.vector.tensor_tensor(
                out=SC[:].rearrange("p (g d) -> p g d", d=64), in0=sc3,
                in1=mx[:].rearrange("p g -> p g ()").to_broadcast(
                    [128, NTILE * 2, 64]),
                op=ALU.add)
            nc.scalar.activation(ATT[:], SC[:], ACT.Exp)
            nc.vector.tensor_reduce(out=sm[:],
                                    in_=ATT[:].rearrange("p (g d) -> p g d",
                                                         d=64),
                                    axis=AX.X, op=ALU.add)
            nc.vector.reciprocal(sm[:], sm[:])
            nc.vector.tensor_tensor(
                out=ATT[:].rearrange("p (g d) -> p g d", d=64),
                in0=ATT[:].rearrange("p (g d) -> p g d", d=64),
                in1=sm[:].rearrange("p g -> p g ()").to_broadcast(
                    [128, NTILE * 2, 64]),
                op=ALU.mult)
            # ctx + o + residual + ln1
            for t in range(NTILE):
                pat = ps_t.tile([128, 128], F16, tag="ptT")
                nc.tensor.transpose(out=pat[:],
                                    in_=ATT[:, t * 128:(t + 1) * 128],
                                    identity=ident[:])
                att = tmp.tile([128, 128], F16, tag="att")
                nc.scalar.activation(att[:], pat[:], ACT.Copy)
                # swapped-half transpose: quadrant (s,hh) lands at
                # partition (1-hh)*64 so s!=hh cases get base s*64
                paw = ps_t.tile([128, 128], F16, tag="ptT")
                nc.tensor.transpose(out=paw[:64, :],
                                    in_=ATT[:, t * 128 + 64:t * 128 + 128],
                                    identity=ident[:])
                nc.tensor.transpose(out=paw[64:, :],
                                    in_=ATT[:, t * 128:t * 128 + 64],
                                    identity=ident[:])
                atw = tmp.tile([128, 128], F16, tag="atw")
                nc.scalar.activation(atw[:], paw[:], ACT.Copy)
                ctx16 = tmp.tile([128, 64], F16, tag="ctx16")
                for hh in range(2):
                    pc = ps_t.tile([128, 32], F32, tag="pt")
                    for s in range(2):
                        src_att = att if s == hh else atw
                        nc.tensor.matmul(
                            out=pc[s * 64:s * 64 + 64, :],
                            lhsT=src_att[s * 64:s * 64 + 64,
                                         s * 64:s * 64 + 64],
                            rhs=VV[s * 64:s * 64 + 64,
                                   t * 64 + hh * 32:t * 64 + hh * 32 + 32],
                            start=True, stop=True)
                    nc.vector.tensor_copy(ctx16[:, hh * 32:hh * 32 + 32],
                                          pc[:])
                ctxT = tmp.tile([64, 128], F16, tag="ctxT")
                pct = ps_t.tile([128, 128], F16, tag="ptT")
                nc.tensor.transpose(out=pct[:64, :], in_=ctx16[:],
                                    identity=ident[:])
                nc.scalar.activation(ctxT[:], pct[:64, :], ACT.Copy)
                po = ps_t.tile([128, 128], F32, tag="pt")
                nc.tensor.matmul(out=po[:, :64], lhsT=ctxT[:], rhs=Wo[i],
                                 start=True, stop=True)
                # h += o + bo
                nc.vector.tensor_tensor(out=po[:, :64], in0=po[:, :64],
                                        in1=bo_r[i], op=ALU.add)
                nc.vector.tensor_tensor(out=h[:, t * 64:(t + 1) * 64],
                                        in0=h[:, t * 64:(t + 1) * 64],
